# revision 1
# baseline (speedup 1.0000x reference)
"""Bass/Tile MHA kernel for TRN2 — per-core program.

Sharding (8 cores): core c handles batch b=c//2, head-group g=c%2
(8 of 16 heads).  Per core:
  x [2048,1024] f32, wq/wk/wv [1024,512] f32 (col slice),
  wo [512,1024] bf16 (row slice), bq/bk [512], bvb [128,512] (bv bcast),
  masks [4,128,512] bf16, ident [128,128] f32.
  out [2048,1024] f32 partial (host sums TP pairs + bo).

Algorithm per core (all matmuls fp32r unless noted):
  xT = x^T          (PE transpose, fp32)
  qT = wq^T-ish: qT[j,s] = sum_d wq[d,j] x[s,d] + bq  (j on partitions)
  kT likewise; v[s,dv] (+ ones col per head, bf16, interleaved 65-stride)
  per head pair hp (2 heads on partition halves of qT/kT chunk):
    scoresT[k,q] = K^T·Q  (row-strip packed, K=64), exp (ACT, ->bf16),
    causal mask on diagonal tiles (mul by 0/1 mask),
    attnV: po[65,512] += [V|1]^T-ish · e  (bf16), denom in row 64,
    normalize: oT[dv,q] = po[0:64]/po[64]  (->bf16)
  D: out[s,od] = sum_dv oT[dv,s]·wo[dv,od]  (bf16 matmul)
"""

from contextlib import ExitStack

import numpy as np

import concourse.bass as bass
import concourse.bacc as bacc
import concourse.mybir as mybir
import concourse.tile as tile

F32 = mybir.dt.float32
F32R = mybir.dt.float32r
BF16 = mybir.dt.bfloat16
ADD = mybir.AluOpType.add
MULT = mybir.AluOpType.mult
EXP = mybir.ActivationFunctionType.Exp

S = 2048          # sequence length
D = 1024          # model dim
JC = 512          # per-core projection width (8 heads * 64)
DK = 64           # head dim
NH = 8            # heads per core
NSC = 4           # s-chunks of 512
NST = 16          # s-tiles of 128
ND = 8            # d-chunks of 128
NJ = 4            # j-chunks of 128 (head pairs)
VW = DK + 1       # 65: even-head attnV window (V + ones col)
PW = 160          # v pair block: [V_even(64)|ones(1)|gap(31)|V_odd(64)]


def build_mha(exp_bufs=2):
    nc = bacc.Bacc("TRN2", target_bir_lowering=False, debug=False)

    x = nc.dram_tensor("x", [S, D], F32, kind="ExternalInput").ap()
    wq = nc.dram_tensor("wq", [D, JC], F32, kind="ExternalInput").ap()
    wk = nc.dram_tensor("wk", [D, JC], F32, kind="ExternalInput").ap()
    wv = nc.dram_tensor("wv", [D, JC], F32, kind="ExternalInput").ap()
    wo = nc.dram_tensor("wo", [JC, D], BF16, kind="ExternalInput").ap()
    bq = nc.dram_tensor("bq", [JC], F32, kind="ExternalInput").ap()
    bk = nc.dram_tensor("bk", [JC], F32, kind="ExternalInput").ap()
    bvb = nc.dram_tensor("bvb", [128, JC], F32, kind="ExternalInput").ap()
    masks = nc.dram_tensor("masks", [128, 896], BF16, kind="ExternalInput").ap()
    ident = nc.dram_tensor("ident", [128, 128], F32, kind="ExternalInput").ap()
    out = nc.dram_tensor("out", [S, D], F32, kind="ExternalOutput").ap()

    with tile.TileContext(nc) as tc, ExitStack() as ctx:
        const = ctx.enter_context(tc.tile_pool(name="const", bufs=1))
        bq_t = const.tile([128, NJ], F32)
        bk_t = const.tile([128, NJ], F32)
        ones_t = const.tile([128, 64], BF16)
        bvb_t = const.tile([128, JC], F32)
        ident_t = const.tile([128, 128], F32)
        mask_t = const.tile([128, 896], BF16)
        nc.sync.dma_start(ident_t[:], ident[:, :])

        # persistent activations
        big = ctx.enter_context(tc.tile_pool(name="big", bufs=1))
        qT_t = big.tile([128, NJ * S], F32R, tag="qT")    # [j, s] 32KB/part
        kT_t = big.tile([128, NJ * S], F32R, tag="kT")    # [j, s] 32KB/part
        v_t = big.tile([128, NST * 4 * PW], BF16, tag="v")  # [s, pair*160] 20KB
        oT_t = big.tile([128, NJ * S], BF16, tag="oT")    # [dv, q] 16KB

        # psum pools (8 banks total: 2 + 2*2 + 2*1)
        pps = ctx.enter_context(tc.tile_pool(name="pps", bufs=2, space="PSUM"))
        psc = ctx.enter_context(tc.tile_pool(name="psc", bufs=2, space="PSUM"))
        pat = ctx.enter_context(tc.tile_pool(name="pat", bufs=2, space="PSUM"))

        ep = ctx.enter_context(tc.tile_pool(name="exp", bufs=exp_bufs))
        rp = ctx.enter_context(tc.tile_pool(name="recip", bufs=2))
        rbp = ctx.enter_context(tc.tile_pool(name="rb", bufs=2))

        wp = ctx.enter_context(tc.tile_pool(name="wts", bufs=1))
        xin = ctx.enter_context(tc.tile_pool(name="xin", bufs=3))
        xTp = ctx.enter_context(tc.tile_pool(name="xT", bufs=1))
        wq_t = wp.tile([128, ND * JC], F32R, tag="wq")   # [d, (dc j)] 16KB
        wk_t = wp.tile([128, ND * JC], F32R, tag="wk")
        wv_t = wp.tile([128, ND * JC], F32R, tag="wv")
        # stage fp32 weights, round into f32r tiles (verifier requires the
        # matmul input's producer to be a rounding compute op, not a DMA)
        with tc.tile_pool(name="wstage", bufs=1) as wsp:
            hd = ND // 2
            for w_dram, w_sb in ((wq, wq_t), (wk, wk_t), (wv, wv_t)):
                wd3 = w_dram.rearrange("(c p) j -> p c j", p=128)
                for half in range(2):
                    wst = wsp.tile([128, hd * JC], F32, tag="wst", name="wst")
                    nc.gpsimd.dma_start(
                        wst[:].rearrange("p (c j) -> p c j", c=hd),
                        wd3[:, half * hd:(half + 1) * hd, :])
                    nc.any.tensor_copy(
                        w_sb[:, half * hd * JC:(half + 1) * hd * JC], wst[:])
        wqr = wq_t[:]
        wkr = wk_t[:]
        wvr = wv_t[:]
        nc.gpsimd.dma_start(bq_t[:], bq.rearrange("(c p) -> p c", p=128))
        nc.gpsimd.dma_start(bk_t[:], bk.rearrange("(c p) -> p c", p=128))
        nc.gpsimd.dma_start(bvb_t[:], bvb[:, :])
        nc.gpsimd.dma_start(mask_t[:], masks[:, :])
        nc.vector.memset(ones_t[:], 1.0)

        def phase_b(sc):
            """Transpose x s-chunk sc, project Q^T/K^T columns + V rows."""
            xT = xTp.tile([128, ND * 512], F32R, tag="xTc")  # [d, (dc s512)]
            xTr = xT[:].rearrange("p (c s) -> p c s", c=ND)
            for st4 in range(4):
                st = sc * 4 + st4
                xi = xin.tile([128, D], F32)
                nc.sync.dma_start(xi[:], x[st * 128:(st + 1) * 128, :])
                for dg in range(2):
                    tp = pps.tile([128, 512], F32, tag="pp")
                    for d4 in range(4):
                        dc = dg * 4 + d4
                        nc.tensor.transpose(
                            tp[:, d4 * 128:(d4 + 1) * 128],
                            xi[:, dc * 128:(dc + 1) * 128], ident_t[:])
                    # scatter 4 d-tiles into xT chunk columns st4*128..
                    dst = xTr[:, dg * 4:(dg + 1) * 4, st4 * 128:(st4 + 1) * 128]
                    src = tp[:].rearrange("p (c s) -> p c s", c=4)
                    nc.vector.tensor_copy(dst, src)
            # Q^T / K^T: psum [128 j, 512 s] accumulated over d-chunks
            for w_r, b_t, dstT in ((wqr, bq_t, qT_t), (wkr, bk_t, kT_t)):
                for jt in range(NJ):
                    pq = pps.tile([128, 512], F32, tag="pp")
                    for dc in range(ND):
                        nc.tensor.matmul(
                            pq[:],
                            w_r[:, dc * JC + jt * 128: dc * JC + (jt + 1) * 128],
                            xTr[:, dc, :],
                            start=(dc == 0), stop=(dc == ND - 1))
                    nc.vector.tensor_scalar(
                        dstT[:, jt * S + sc * 512: jt * S + (sc + 1) * 512],
                        pq[:], b_t[:, jt:jt + 1], None, op0=ADD)
            # V rows: psum [128 s, 512 dv] -> pair-block layout in v_t
            vr = v_t[:].rearrange("p (s q w) -> p s q w", s=NST, q=4)
            bvr = bvb_t[:].rearrange("p (q two w) -> p q two w", q=4, two=2)
            for st4 in range(4):
                st = sc * 4 + st4
                pv = pps.tile([128, 512], F32, tag="pp")
                for dc in range(ND):
                    nc.tensor.matmul(
                        pv[:],
                        xTr[:, dc, st4 * 128:(st4 + 1) * 128],
                        wvr[:, dc * JC:(dc + 1) * JC],
                        start=(dc == 0), stop=(dc == ND - 1))
                pv4 = pv[:].rearrange("p (q two w) -> p q two w", q=4, two=2)
                nc.any.tensor_tensor(
                    vr[:, st, :, 0:DK], pv4[:, :, 0, :], bvr[:, :, 0, :], op=ADD)
                nc.any.tensor_tensor(
                    vr[:, st, :, 96:96 + DK], pv4[:, :, 1, :], bvr[:, :, 1, :],
                    op=ADD)
                nc.any.memset(vr[:, st, :, DK:DK + 1], 1.0)
                nc.any.memset(vr[:, st, :, DK + 1:96], 0.0)

        def phase_c(hp, qt):
            """Attention for head pair hp, q-chunk qt (512 wide).

            Even head (h2=0): attnV lhsT = [V_h|ones] (65 wide) -> po rows
            0-63 numerator, row 64 denominator.  Odd head (h2=1): lhsT is
            the shifted 128-wide window [junk63|ones|V_h] so the numerator
            lands on partitions 64-127 (matching oT) and the denominator on
            partition 63; rows 0-62 are garbage and never read.
            """
            n_k = (qt + 1) * 4
            qcol = hp * S + qt * 512
            po = [pat.tile([VW, 512], F32, tag="po", name="po0"),
                  pat.tile([128, 512], F32, tag="po", name="po1")]
            for kg in range(n_k // 2):
                ps2 = [psc.tile([128, 1024], F32, tag="sc", name=f"sc{h2}")
                       for h2 in range(2)]
                for k2 in range(2):
                    kt = kg * 2 + k2
                    for h2 in range(2):
                        lo = h2 * 64
                        nc.tensor.matmul(
                            ps2[h2][:, k2 * 512:(k2 + 1) * 512],
                            kT_t[lo:lo + 64, hp * S + kt * 128: hp * S + (kt + 1) * 128],
                            qT_t[lo:lo + 64, qcol: qcol + 512],
                            start=True, stop=True)
                for h2 in range(2):
                    h = hp * 2 + h2
                    e = ep.tile([128, 1024], BF16, tag="e")
                    nc.scalar.activation(e[:], ps2[h2][:], EXP, scale=0.125)
                    for k2 in range(2):
                        kt = kg * 2 + k2
                        dd = kt - qt * 4  # diagonal offset index
                        ekt = e[:, k2 * 512:(k2 + 1) * 512]
                        if dd >= 0:
                            off = 384 - dd * 128
                            nc.any.tensor_tensor(
                                ekt, ekt, mask_t[:, off:off + 512], op=MULT)
                        base = kt * 4 * PW + hp * PW
                        if h2 == 0:
                            lhsT = v_t[:, base: base + VW]
                        else:
                            lhsT = v_t[:, base + 32: base + 160]
                        nc.tensor.matmul(
                            po[h2][:], lhsT, ekt,
                            start=(kt == 0), stop=(kt == n_k - 1))
            for h2 in range(2):
                dp = 64 if h2 == 0 else 32       # denominator partition
                lo = h2 * 64
                rec = rp.tile([VW, 512], BF16, tag="rec", name="rec")
                with nc.allow_low_precision(reason="bf16 recip for PE bcast"):
                    nc.vector.reciprocal(rec[dp:dp + 1, :], po[h2][dp:dp + 1, :])
                # broadcast along partitions via K=1 ones matmul on the PE
                rb_ps = psc.tile([128, 512], F32, tag="sc", name="rbps")
                nc.tensor.matmul(rb_ps[lo:lo + 64, :], ones_t[dp:dp + 1, :],
                                 rec[dp:dp + 1, :], start=True, stop=True)
                rb = rbp.tile([128, 512], BF16, tag="rb", name="rb")
                nc.vector.tensor_copy(rb[lo:lo + 64, :], rb_ps[lo:lo + 64, :])
                nc.any.tensor_tensor(
                    oT_t[lo:lo + 64, qcol: qcol + 512],
                    po[h2][lo:lo + 64, :], rb[lo:lo + 64, :], op=MULT)

        # phase D body: out[s, od] = sum_dv oT[dv, s] * wo[dv, od]
        wop = ctx.enter_context(tc.tile_pool(name="wo", bufs=1))
        ostp = ctx.enter_context(tc.tile_pool(name="ost", bufs=2))
        wo_t = wop.tile([128, NJ * D], BF16)
        nc.gpsimd.dma_start(wo_t[:].rearrange("p (c j) -> p c j", c=NJ),
                            wo.rearrange("(c p) j -> p c j", p=128))

        def phase_d(sc):
            for st in range(sc * 4, sc * 4 + 4):
                ot = ostp.tile([128, D], F32, tag="ost", name="ot")
                for od in range(2):
                    pd = pps.tile([128, 512], F32, tag="pp", name="pd")
                    for vc in range(NJ):
                        nc.tensor.matmul(
                            pd[:],
                            oT_t[:, vc * S + st * 128: vc * S + (st + 1) * 128],
                            wo_t[:, vc * D + od * 512: vc * D + (od + 1) * 512],
                            start=(vc == 0), stop=(vc == NJ - 1))
                    nc.vector.tensor_copy(ot[:, od * 512:(od + 1) * 512], pd[:])
                nc.sync.dma_start(out[st * 128:(st + 1) * 128, :], ot[:])

        # B/C diagonal interleave: C(·, qt) only needs s-chunks <= qt
        for sc in range(NSC):
            phase_b(sc)
            for hp in range(NJ):
                phase_c(hp, sc)
        for sc in range(NSC):
            phase_d(sc)

    nc.compile()
    return nc


# ----------------------------------------------------------------- host side

_NC_CACHE = None


def _get_nc():
    global _NC_CACHE
    if _NC_CACHE is None:
        _NC_CACHE = build_mha()
    return _NC_CACHE


def make_masks():
    i = np.arange(128)[:, None]
    jj = np.arange(896)[None, :]
    return (i + 384 <= jj).astype(np.float32)


def shard_inputs(x, Wq, bq, Wk, bk, Wv, bv, Wo, bo):
    import ml_dtypes
    masks = make_masks().astype(ml_dtypes.bfloat16)
    ident = np.eye(128, dtype=np.float32)
    x = np.asarray(x, dtype=np.float32)
    maps = []
    for c in range(8):
        b, g = c // 2, c % 2
        sl = slice(g * JC, (g + 1) * JC)
        maps.append({
            "x": np.ascontiguousarray(x[b]),
            "wq": np.ascontiguousarray(np.asarray(Wq)[:, sl]),
            "wk": np.ascontiguousarray(np.asarray(Wk)[:, sl]),
            "wv": np.ascontiguousarray(np.asarray(Wv)[:, sl]),
            "wo": np.ascontiguousarray(np.asarray(Wo)[sl, :]).astype(
                ml_dtypes.bfloat16),
            "bq": np.ascontiguousarray(np.asarray(bq)[sl]),
            "bk": np.ascontiguousarray(np.asarray(bk)[sl]),
            "bvb": np.broadcast_to(np.asarray(bv)[sl], (128, JC)).copy(),
            "masks": masks,
            "ident": ident,
        })
    return maps


def kernel(x, Wq, bq, Wk, bk, Wv, bv, Wo, bo):
    """Full-input entry point: shard across 8 NeuronCores, run, gather."""
    from concourse.bass_utils import run_bass_kernel_spmd

    nc = _get_nc()
    in_maps = shard_inputs(x, Wq, bq, Wk, bk, Wv, bv, Wo, bo)
    res = run_bass_kernel_spmd(nc, in_maps, list(range(8)))
    bo = np.asarray(bo, dtype=np.float32)
    out = np.empty((4, S, D), dtype=np.float32)
    for b in range(4):
        out[b] = res.results[2 * b]["out"] + res.results[2 * b + 1]["out"] + bo
    return out



# revision 2
# speedup vs baseline: 1.1007x; 1.1007x over previous
"""Bass/Tile MHA kernel for TRN2 — per-core program (v2).

Sharding (8 cores): core c handles batch b=c//2, head-group g=c%2
(8 of 16 heads).  Host pre-transposes x per batch to xT [1024, 2048]
bf16, slices weights per head group to bf16, folds bv into
bo' = bo + bv @ Wo (softmax weights sum to 1), and drops bk entirely
(a key-side bias adds a per-query constant to every logit, which
softmax cancels).

Per core inputs:
  xT [1024,2048] bf16, wq/wk/wv [1024,512] bf16 (col slice),
  wo [512,1024] bf16 (row slice), bqt [128,4] f32 (bq per j-tile),
  masks [128,512] bf16 (diag dd=0 | dd=128 for 256-wide q chunks).
  out [2048,1024] f32 partial (host sums TP pairs + bo').

Per-core schedule (all matmuls bf16, 1 cyc/row):
  B(sc): DMA xT s-chunk -> qT[j,s] (+bq, bf16), kT[j,s], v pair blocks
  C(hp,qc): per head pair, 256-wide q chunk: scores K^T.Q per 128-k
    tile (only causal tiles), exp (ACT -> bf16), mask diag tiles,
    attnV accumulate [V|1]^T.e, normalize -> oT[dv,q] bf16
  D(st): out[s,:] = sum_dv oT[dv,s].wo[dv,:]
  Loop interleaves B(sc+1) and D(sc-1) pieces into C's ACT-bound
  stretches so the PE never starves.
"""

from contextlib import ExitStack

import numpy as np

import concourse.bass as bass
import concourse.bacc as bacc
import concourse.mybir as mybir
import concourse.tile as tile

F32 = mybir.dt.float32
BF16 = mybir.dt.bfloat16
ADD = mybir.AluOpType.add
MULT = mybir.AluOpType.mult
EXP = mybir.ActivationFunctionType.Exp

S = 2048          # sequence length
D = 1024          # model dim
JC = 512          # per-core projection width (8 heads * 64)
DK = 64           # head dim
NSC = 4           # s-chunks of 512
NST = 16          # s-tiles of 128
ND = 8            # d-chunks of 128
NJ = 4            # j-chunks of 128 (head pairs)
NQC = 8           # q-chunks of 256
QW = 256          # q-chunk width in phase C
VW = DK + 1       # 65: even-head attnV window (V + ones col)
PW = 160          # v pair block: [V_even(64)|ones(1)|gap(31)|V_odd(64)]


def build_mha():
    nc = bacc.Bacc("TRN2", target_bir_lowering=False, debug=False)

    xT = nc.dram_tensor("xT", [D, S], BF16, kind="ExternalInput").ap()
    wq = nc.dram_tensor("wq", [D, JC], BF16, kind="ExternalInput").ap()
    wk = nc.dram_tensor("wk", [D, JC], BF16, kind="ExternalInput").ap()
    wv = nc.dram_tensor("wv", [D, JC], BF16, kind="ExternalInput").ap()
    wo = nc.dram_tensor("wo", [JC, D], BF16, kind="ExternalInput").ap()
    bqt = nc.dram_tensor("bqt", [128, NJ], F32, kind="ExternalInput").ap()
    masks = nc.dram_tensor("masks", [128, 2 * QW], BF16,
                           kind="ExternalInput").ap()
    out = nc.dram_tensor("out", [S, D], F32, kind="ExternalOutput").ap()

    xTd3 = xT.rearrange("(c p) s -> p c s", p=128)

    with tile.TileContext(nc) as tc, ExitStack() as ctx:
        const = ctx.enter_context(tc.tile_pool(name="const", bufs=1))
        bq_t = const.tile([128, NJ], F32)
        ones_t = const.tile([128, DK], BF16)
        mask_t = const.tile([128, 2 * QW], BF16)

        # persistent activations
        big = ctx.enter_context(tc.tile_pool(name="big", bufs=1))
        qT_t = big.tile([128, NJ * S], BF16, tag="qT")    # [j, s] 16KB/part
        kT_t = big.tile([128, NJ * S], BF16, tag="kT")    # [j, s] 16KB/part
        v_t = big.tile([128, NST * 4 * PW], BF16, tag="v")  # [s, pair*160]
        oT_t = big.tile([128, NJ * S], BF16, tag="oT")    # [dv, q] 16KB

        # psum pools (8 banks: pps 2*1 + psc 2*2 + pat 2*1)
        pps = ctx.enter_context(tc.tile_pool(name="pps", bufs=2, space="PSUM"))
        psc = ctx.enter_context(tc.tile_pool(name="psc", bufs=2, space="PSUM"))
        pat = ctx.enter_context(tc.tile_pool(name="pat", bufs=2, space="PSUM"))

        ep = ctx.enter_context(tc.tile_pool(name="exp", bufs=3))
        rp = ctx.enter_context(tc.tile_pool(name="recip", bufs=2))
        rbp = ctx.enter_context(tc.tile_pool(name="rb", bufs=2))

        wp = ctx.enter_context(tc.tile_pool(name="wts", bufs=1))
        xin = ctx.enter_context(tc.tile_pool(name="xin", bufs=2))
        wq_t = wp.tile([128, ND * JC], BF16, tag="wq")   # [d, (dc j)] 8KB
        wk_t = wp.tile([128, ND * JC], BF16, tag="wk")
        wv_t = wp.tile([128, ND * JC], BF16, tag="wv")
        wo_t = wp.tile([128, NJ * D], BF16, tag="wo")

        ostp = ctx.enter_context(tc.tile_pool(name="ost", bufs=2))

        xc_tiles = {}

        def dma_xc(sc):
            xc = xin.tile([128, ND * 512], BF16, tag="xc", name="xc")
            nc.sync.dma_start(
                xc[:].rearrange("p (c s) -> p c s", c=ND),
                xTd3[:, :, sc * 512:(sc + 1) * 512])
            xc_tiles[sc] = xc[:].rearrange("p (c s) -> p c s", c=ND)

        # ---- startup DMAs (emission order = fetch priority) ----
        dma_xc(0)
        nc.sync.dma_start(wq_t[:].rearrange("p (c j) -> p c j", c=ND),
                          wq.rearrange("(c p) j -> p c j", p=128))
        nc.gpsimd.dma_start(bq_t[:], bqt[:, :])
        nc.sync.dma_start(wk_t[:].rearrange("p (c j) -> p c j", c=ND),
                          wk.rearrange("(c p) j -> p c j", p=128))
        nc.sync.dma_start(wv_t[:].rearrange("p (c j) -> p c j", c=ND),
                          wv.rearrange("(c p) j -> p c j", p=128))
        nc.gpsimd.dma_start(mask_t[:], masks[:, :])
        nc.sync.dma_start(wo_t[:].rearrange("p (c j) -> p c j", c=NJ),
                          wo.rearrange("(c p) j -> p c j", p=128))
        nc.vector.memset(ones_t[:], 1.0)

        vr = v_t[:].rearrange("p (s q w) -> p s q w", s=NST, q=4)
        nc.vector.memset(vr[:, :, :, DK:DK + 1], 1.0)
        nc.vector.memset(vr[:, :, :, DK + 1:96], 0.0)

        # ---- phase B pieces: 12 matmul groups per s-chunk ----
        def b_group(sc, gi):
            """Group gi in 0..11: 0-3 qT j-tiles, 4-7 kT j-tiles, 8-11 V."""
            xcr = xc_tiles[sc]
            if gi < 8:
                w_t, dstT = (wq_t, qT_t) if gi < 4 else (wk_t, kT_t)
                jt = gi % 4
                pq = pps.tile([128, 512], F32, tag="pp", name="pq")
                for dc in range(ND):
                    nc.tensor.matmul(
                        pq[:],
                        w_t[:, dc * JC + jt * 128: dc * JC + (jt + 1) * 128],
                        xcr[:, dc, :],
                        start=(dc == 0), stop=(dc == ND - 1))
                dst = dstT[:, jt * S + sc * 512: jt * S + (sc + 1) * 512]
                if gi < 4:
                    nc.vector.tensor_scalar(
                        dst, pq[:], bq_t[:, jt:jt + 1], None, op0=ADD)
                else:
                    nc.vector.tensor_copy(dst, pq[:])
            else:
                st4 = gi - 8
                st = sc * 4 + st4
                pv = pps.tile([128, 512], F32, tag="pp", name="pv")
                for dc in range(ND):
                    nc.tensor.matmul(
                        pv[:],
                        xcr[:, dc, st4 * 128:(st4 + 1) * 128],
                        wv_t[:, dc * JC:(dc + 1) * JC],
                        start=(dc == 0), stop=(dc == ND - 1))
                pv4 = pv[:].rearrange("p (q two w) -> p q two w", q=4, two=2)
                nc.vector.tensor_copy(vr[:, st, :, 0:DK], pv4[:, :, 0, :])
                nc.vector.tensor_copy(vr[:, st, :, 96:96 + DK],
                                      pv4[:, :, 1, :])

        def phase_c(hp, qc):
            """Attention for head pair hp, q-chunk qc (256 wide)."""
            n_k = 2 * qc + 2
            qcol = hp * S + qc * QW
            po = [pat.tile([VW, QW], F32, tag="po", name="po0"),
                  pat.tile([128, QW], F32, tag="po", name="po1")]
            for k0 in range(0, n_k, 4):
                ng = min(4, n_k - k0)
                for h2 in range(2):
                    lo = h2 * 64
                    ps = psc.tile([128, 1024], F32, tag="sc",
                                  name=f"sc{h2}")
                    for i in range(ng):
                        kt = k0 + i
                        nc.tensor.matmul(
                            ps[:, i * QW:(i + 1) * QW],
                            kT_t[lo:lo + 64,
                                 hp * S + kt * 128: hp * S + (kt + 1) * 128],
                            qT_t[lo:lo + 64, qcol: qcol + QW],
                            start=True, stop=True)
                    e = ep.tile([128, 1024], BF16, tag="e", name="e")
                    nc.scalar.activation(e[:, :ng * QW], ps[:, :ng * QW],
                                         EXP, scale=0.125)
                    for i in range(ng):
                        kt = k0 + i
                        ekt = e[:, i * QW:(i + 1) * QW]
                        if kt >= n_k - 2:  # diagonal tiles need masking
                            off = QW if kt == n_k - 1 else 0
                            nc.vector.tensor_tensor(
                                ekt, ekt, mask_t[:, off:off + QW], op=MULT)
                        base = kt * 4 * PW + hp * PW
                        if h2 == 0:
                            lhsT = v_t[:, base: base + VW]
                        else:
                            lhsT = v_t[:, base + 32: base + 160]
                        nc.tensor.matmul(
                            po[h2][:], lhsT, ekt,
                            start=(kt == 0), stop=(kt == n_k - 1))
            for h2 in range(2):
                dp = 64 if h2 == 0 else 32       # denominator partition
                lo = h2 * 64
                rec = rp.tile([VW, QW], BF16, tag="rec", name="rec")
                with nc.allow_low_precision(reason="bf16 recip for PE bcast"):
                    nc.vector.reciprocal(rec[dp:dp + 1, :],
                                         po[h2][dp:dp + 1, :])
                rb_ps = psc.tile([128, QW], F32, tag="sc", name="rbps")
                nc.tensor.matmul(rb_ps[lo:lo + 64, :], ones_t[dp:dp + 1, :],
                                 rec[dp:dp + 1, :], start=True, stop=True)
                rb = rbp.tile([128, QW], BF16, tag="rb", name="rb")
                nc.vector.tensor_copy(rb[lo:lo + 64, :], rb_ps[lo:lo + 64, :])
                nc.vector.tensor_tensor(
                    oT_t[lo:lo + 64, qcol: qcol + QW],
                    po[h2][lo:lo + 64, :], rb[lo:lo + 64, :], op=MULT)

        def phase_d(st):
            ot = ostp.tile([128, D], F32, tag="ost", name="ot")
            for od in range(2):
                pd = pps.tile([128, 512], F32, tag="pp", name="pd")
                for vc in range(NJ):
                    nc.tensor.matmul(
                        pd[:],
                        oT_t[:, vc * S + st * 128: vc * S + (st + 1) * 128],
                        wo_t[:, vc * D + od * 512: vc * D + (od + 1) * 512],
                        start=(vc == 0), stop=(vc == NJ - 1))
                nc.vector.tensor_copy(ot[:, od * 512:(od + 1) * 512], pd[:])
            nc.sync.dma_start(out[st * 128:(st + 1) * 128, :], ot[:])

        # ---- main loop: C(., sc) interleaved with B(sc+1) + D(sc-1) ----
        for gi in range(12):
            b_group(0, gi)
        for sc in range(NSC):
            for hp in range(NJ):
                phase_c(hp, 2 * sc)
                phase_c(hp, 2 * sc + 1)
                if sc < NSC - 1:
                    if hp == 0:
                        dma_xc(sc + 1)
                    for gi in range(3 * hp, 3 * hp + 3):
                        b_group(sc + 1, gi)
                if sc > 0:
                    phase_d((sc - 1) * 4 + hp)
        for st4 in range(4):
            phase_d(12 + st4)

    nc.compile()
    return nc


# ----------------------------------------------------------------- host side

_NC_CACHE = None


def _get_nc():
    global _NC_CACHE
    if _NC_CACHE is None:
        _NC_CACHE = build_mha()
    return _NC_CACHE


def make_masks():
    """[128, 512]: two diagonal masks for [128k x 256q] tiles.
    mask0: k-tile aligned with q-chunk start (keep k<=q: i<=j).
    mask128: k-tile offset +128 (keep i+128<=j)."""
    i = np.arange(128)[:, None]
    j = np.arange(QW)[None, :]
    m0 = (i <= j).astype(np.float32)
    m128 = (i + 128 <= j).astype(np.float32)
    return np.concatenate([m0, m128], axis=1)


def shard_inputs(x, Wq, bq, Wk, bk, Wv, bv, Wo, bo):
    import ml_dtypes
    masks = make_masks().astype(ml_dtypes.bfloat16)
    x = np.asarray(x, dtype=np.float32)
    Wq, Wk, Wv, Wo = (np.asarray(a, dtype=np.float32)
                      for a in (Wq, Wk, Wv, Wo))
    bq = np.asarray(bq, dtype=np.float32)
    maps = []
    for c in range(8):
        b, g = c // 2, c % 2
        sl = slice(g * JC, (g + 1) * JC)
        # bq per-core slice laid out [128 part, jt]: j = jt*128 + p
        bqt = np.ascontiguousarray(
            bq[sl].reshape(NJ, 128).T).astype(np.float32)
        maps.append({
            "xT": np.ascontiguousarray(x[b].T).astype(ml_dtypes.bfloat16),
            "wq": np.ascontiguousarray(Wq[:, sl]).astype(ml_dtypes.bfloat16),
            "wk": np.ascontiguousarray(Wk[:, sl]).astype(ml_dtypes.bfloat16),
            "wv": np.ascontiguousarray(Wv[:, sl]).astype(ml_dtypes.bfloat16),
            "wo": np.ascontiguousarray(Wo[sl, :]).astype(ml_dtypes.bfloat16),
            "bqt": bqt,
            "masks": masks,
        })
    return maps


def kernel(x, Wq, bq, Wk, bk, Wv, bv, Wo, bo):
    """Full-input entry point: shard across 8 NeuronCores, run, gather."""
    from concourse.bass_utils import run_bass_kernel_spmd

    nc = _get_nc()
    in_maps = shard_inputs(x, Wq, bq, Wk, bk, Wv, bv, Wo, bo)
    res = run_bass_kernel_spmd(nc, in_maps, list(range(8)))
    # bv contributes bv @ Wo to every output row (softmax weights sum to 1)
    bo_eff = (np.asarray(bo, dtype=np.float32)
              + np.asarray(bv, dtype=np.float32)
              @ np.asarray(Wo, dtype=np.float32))
    out = np.empty((4, S, D), dtype=np.float32)
    for b in range(4):
        out[b] = res.results[2 * b]["out"] + res.results[2 * b + 1]["out"] \
            + bo_eff
    return out


# revision 29
# speedup vs baseline: 1.2810x; 1.1638x over previous
"""Bass/Tile MHA kernel for TRN2 — per-core program (v2).

Sharding (8 cores): core c handles batch b=c//2, head-group g=c%2
(8 of 16 heads).  Host pre-transposes x per batch to xT [1024, 2048]
bf16, slices weights per head group to bf16, folds bv into
bo' = bo + bv @ Wo (softmax weights sum to 1), and drops bk entirely
(a key-side bias adds a per-query constant to every logit, which
softmax cancels).

Per core inputs:
  xT [1024,2048] bf16, wq/wk/wv [1024,512] bf16 (col slice),
  wo [512,1024] bf16 (row slice), bqt [128,4] f32 (bq per j-tile),
  masks [128,512] bf16 (diag dd=0 | dd=128 for 256-wide q chunks).
  out [2048,1024] f32 partial (host sums TP pairs + bo').

Per-core schedule (all matmuls bf16, 1 cyc/row):
  B(sc): DMA xT s-chunk -> qT[j,s] (+bq, bf16), kT[j,s], v pair blocks
  C(hp,qc): per head pair, 256-wide q chunk: scores K^T.Q per 128-k
    tile (only causal tiles), exp (ACT -> bf16), mask diag tiles,
    attnV accumulate [V|1]^T.e, normalize -> oT[dv,q] bf16
  D(st): out[s,:] = sum_dv oT[dv,s].wo[dv,:]
  Loop interleaves B(sc+1) and D(sc-1) pieces into C's ACT-bound
  stretches so the PE never starves.
"""

from contextlib import ExitStack

import numpy as np

import concourse.bass as bass
import concourse.bacc as bacc
import concourse.mybir as mybir
import concourse.tile as tile

F32 = mybir.dt.float32
BF16 = mybir.dt.bfloat16
ADD = mybir.AluOpType.add
MULT = mybir.AluOpType.mult
EXP = mybir.ActivationFunctionType.Exp

S = 2048          # sequence length
D = 1024          # model dim
JC = 512          # per-core projection width (8 heads * 64)
DK = 64           # head dim
NSC = 4           # s-chunks of 512
NST = 16          # s-tiles of 128
ND = 8            # d-chunks of 128
NJ = 4            # j-chunks of 128 (head pairs)
NQC = 8           # q-chunks of 256
QW = 256          # q-chunk width in phase C
VW = DK + 1       # 65: even-head attnV window (V + ones col)
PW = 160          # v pair block: [V_even(64)|ones(1)|gap(31)|V_odd(64)]


def build_mha():
    nc = bacc.Bacc("TRN2", target_bir_lowering=False, debug=False)

    xT = nc.dram_tensor("xT", [D, S], BF16, kind="ExternalInput").ap()
    wq = nc.dram_tensor("wq", [D, JC], BF16, kind="ExternalInput").ap()
    wk = nc.dram_tensor("wk", [D, JC], BF16, kind="ExternalInput").ap()
    wv = nc.dram_tensor("wv", [D, JC], BF16, kind="ExternalInput").ap()
    wo = nc.dram_tensor("wo", [JC, D], BF16, kind="ExternalInput").ap()
    bqt = nc.dram_tensor("bqt", [128, NJ], F32, kind="ExternalInput").ap()
    masks = nc.dram_tensor("masks", [128, 2 * QW], BF16,
                           kind="ExternalInput").ap()
    ident = nc.dram_tensor("ident", [128, 128], BF16,
                           kind="ExternalInput").ap()
    out = nc.dram_tensor("out", [S, D], F32, kind="ExternalOutput").ap()

    xTd3 = xT.rearrange("(c p) s -> p c s", p=128)

    with tile.TileContext(nc) as tc, ExitStack() as ctx:
        const = ctx.enter_context(tc.tile_pool(name="const", bufs=1))
        bq_t = const.tile([128, NJ], F32)
        ident_t = const.tile([128, 128], BF16)
        mask_t = const.tile([128, 2 * QW], BF16)

        # persistent activations
        big = ctx.enter_context(tc.tile_pool(name="big", bufs=1))
        qT_t = big.tile([128, NJ * S], BF16, tag="qT")    # [j, s] 16KB/part
        kT_t = big.tile([128, NJ * S], BF16, tag="kT")    # [j, s] 16KB/part
        v_t = big.tile([128, NST * 4 * PW], BF16, tag="v")  # [s, pair*160]
        o_sb = big.tile([128, NST * NJ * 128], BF16, tag="o")  # [q,(qt hp d)]

        # psum pools (8 banks: pps 2*1 + psc 2*2 + pat 2*1)
        pps = ctx.enter_context(tc.tile_pool(name="pps", bufs=2, space="PSUM"))
        psc = ctx.enter_context(tc.tile_pool(name="psc", bufs=2, space="PSUM"))
        pat = ctx.enter_context(tc.tile_pool(name="pat", bufs=2, space="PSUM"))

        ep = ctx.enter_context(tc.tile_pool(name="exp", bufs=3))
        rp = ctx.enter_context(tc.tile_pool(name="recip", bufs=4))
        otp = ctx.enter_context(tc.tile_pool(name="otp", bufs=2))

        wp = ctx.enter_context(tc.tile_pool(name="wts", bufs=1))
        xin = ctx.enter_context(tc.tile_pool(name="xin", bufs=2))
        wq_t = wp.tile([128, ND * JC], BF16, tag="wq")   # [d, (dc j)] 8KB
        wk_t = wp.tile([128, ND * JC], BF16, tag="wk")
        wv_t = wp.tile([128, ND * JC], BF16, tag="wv")
        wo_t = wp.tile([128, NJ * D], BF16, tag="wo")

        ostp = ctx.enter_context(tc.tile_pool(name="ost", bufs=4))

        xc_tiles = {}

        def dma_xc(sc, half=None):
            if half is None or half == 0:
                xc = xin.tile([128, ND * 512], BF16, tag="xc", name="xc")
                xc_tiles[sc] = xc[:].rearrange("p (c s) -> p c s", c=ND)
            hd = ND // 2
            xcr = xc_tiles[sc]
            halves = range(2) if half is None else (half,)
            for h in halves:
                nc.sync.dma_start(
                    xcr[:, h * hd:(h + 1) * hd, :],
                    xTd3[:, h * hd:(h + 1) * hd, sc * 512:(sc + 1) * 512])

        def dma_w(w_dram, w_sb, half):
            hd = ND // 2
            nc.sync.dma_start(
                w_sb[:].rearrange("p (c j) -> p c j", c=ND)[
                    :, half * hd:(half + 1) * hd, :],
                w_dram.rearrange("(c p) j -> p c j", p=128)[
                    :, half * hd:(half + 1) * hd, :])

        # ---- startup DMAs (emission order = fetch priority) ----
        dma_xc(0, half=0)
        dma_w(wq, wq_t, 0)
        dma_xc(0, half=1)
        dma_w(wq, wq_t, 1)
        dma_w(wk, wk_t, 0)
        dma_w(wk, wk_t, 1)
        dma_w(wv, wv_t, 0)
        dma_w(wv, wv_t, 1)
        nc.gpsimd.dma_start(bq_t[:], bqt[:, :])
        nc.gpsimd.dma_start(mask_t[:], masks[:, :])
        nc.gpsimd.dma_start(ident_t[:], ident[:, :])
        nc.sync.dma_start(wo_t[:].rearrange("p (c j) -> p c j", c=NJ),
                          wo.rearrange("(c p) j -> p c j", p=128))

        # v pair block: [V_even(64) | ones(1) | V_odd(64) | pad(31)]
        vr = v_t[:].rearrange("p (s q w) -> p s q w", s=NST, q=4)
        nc.vector.memset(vr[:, :, :, DK:DK + 1], 1.0)

        # ---- phase B pieces: 12 matmul groups per s-chunk ----
        def b_group(sc, gi):
            """Group gi in 0..11: 0-3 qT j-tiles, 4-7 kT j-tiles, 8-11 V."""
            xcr = xc_tiles[sc]
            if gi < 8:
                w_t, dstT = (wq_t, qT_t) if gi < 4 else (wk_t, kT_t)
                jt = gi % 4
                pq = pps.tile([128, 512], F32, tag="pp", name="pq")
                for dc in range(ND):
                    nc.tensor.matmul(
                        pq[:],
                        w_t[:, dc * JC + jt * 128: dc * JC + (jt + 1) * 128],
                        xcr[:, dc, :],
                        start=(dc == 0), stop=(dc == ND - 1))
                dst = dstT[:, jt * S + sc * 512: jt * S + (sc + 1) * 512]
                if gi < 4:
                    nc.vector.tensor_scalar(
                        dst, pq[:], bq_t[:, jt:jt + 1], None, op0=ADD)
                else:
                    nc.vector.tensor_copy(dst, pq[:])
            else:
                st4 = gi - 8
                st = sc * 4 + st4
                pv = pps.tile([128, 512], F32, tag="pp", name="pv")
                for dc in range(ND):
                    nc.tensor.matmul(
                        pv[:],
                        xcr[:, dc, st4 * 128:(st4 + 1) * 128],
                        wv_t[:, dc * JC:(dc + 1) * JC],
                        start=(dc == 0), stop=(dc == ND - 1))
                pv4 = pv[:].rearrange("p (q two w) -> p q two w", q=4, two=2)
                nc.vector.tensor_copy(vr[:, st, :, 0:DK], pv4[:, :, 0, :])
                nc.vector.tensor_copy(vr[:, st, :, VW:VW + DK],
                                      pv4[:, :, 1, :])

        fillers = []
        fill_state = {"site": 0, "stride": 1}

        def pop_filler(force=False):
            if not fillers:
                return
            if force:
                fillers.pop(0)()
                return
            fill_state["site"] += 1
            if fill_state["site"] % fill_state["stride"] == 0:
                fillers.pop(0)()

        def phase_c(hp, qc):
            """Attention for head pair hp, q-chunk qc (256 wide).

            Scores land as e[k, q]; attnV is flipped: out po[q, V|1] with q
            on the full 128 partitions (lhsT = e 128-q slice, rhs =
            [V|ones] 65-wide moving).  The softmax denominator then lands
            on the same partition as its numerators, so normalization is a
            per-partition reciprocal+scale — no PE broadcast needed.
            """
            n_k = 2 * qc + 2
            for h2 in range(2):
                lo = h2 * 64
                po = [pat.tile([128, VW], F32, tag="po", name=f"po{qs}")
                      for qs in range(2)]
                for k0 in range(0, n_k, 4):
                    if k0 > 0:
                        pop_filler()
                    ng = min(4, n_k - k0)
                    ps = psc.tile([128, 1024], F32, tag="sc", name="ps")
                    for i in range(ng):
                        kt = k0 + i
                        nc.tensor.matmul(
                            ps[:, i * QW:(i + 1) * QW],
                            kT_t[lo:lo + 64,
                                 hp * S + kt * 128: hp * S + (kt + 1) * 128],
                            qT_t[lo:lo + 64,
                                 hp * S + qc * QW: hp * S + (qc + 1) * QW],
                            start=True, stop=True)
                    e = ep.tile([128, 1024], BF16, tag="e", name="e")
                    nc.scalar.activation(e[:, :ng * QW], ps[:, :ng * QW],
                                         EXP, scale=0.125)
                    for i in range(ng):
                        kt = k0 + i
                        ekt = e[:, i * QW:(i + 1) * QW]
                        if kt >= n_k - 2:  # diagonal tiles need masking
                            off = QW if kt == n_k - 1 else 0
                            nc.vector.tensor_tensor(
                                ekt, ekt, mask_t[:, off:off + QW], op=MULT)
                        base = kt * 4 * PW + hp * PW + h2 * DK
                        for qs in range(2):
                            nc.tensor.matmul(
                                po[qs][:],
                                e[:, i * QW + qs * 128: i * QW + qs * 128
                                  + 128],
                                v_t[:, base: base + VW],
                                start=(kt == 0), stop=(kt == n_k - 1))
                # normalize: denominator is col DK (h0) / col 0 (h1)
                dcol, ncol = (DK, 0) if h2 == 0 else (0, 1)
                for qs in range(2):
                    qt = qc * 2 + qs
                    rc = rp.tile([128, 1], F32, tag="rec", name="rc")
                    nc.vector.reciprocal(rc[:], po[qs][:, dcol:dcol + 1])
                    nc.vector.tensor_scalar(
                        o_sb[:, (qt * NJ + hp) * 128 + lo:
                             (qt * NJ + hp) * 128 + lo + DK],
                        po[qs][:, ncol:ncol + DK], rc[:], None, op0=MULT)

        d_tiles = {}

        def phase_d(st, od=None):
            if od is None:
                phase_d(st, 0)
                phase_d(st, 1)
                return
            if od == 0:
                ot = ostp.tile([128, D], F32, tag="ost", name="ot")
                # transpose o[q, dv2] -> oT[dv2, q] per head pair (PE,
                # bf16 identity: 1 cyc/row)
                oTst = otp.tile([128, NJ * 128], BF16, tag="oTst",
                                name="oTst")
                for hp in range(NJ):
                    tp = psc.tile([128, 128], BF16, tag="sc", name="tp")
                    nc.tensor.transpose(
                        tp[:],
                        o_sb[:, (st * NJ + hp) * 128:
                             (st * NJ + hp + 1) * 128],
                        ident_t[:])
                    nc.vector.tensor_copy(oTst[:, hp * 128:(hp + 1) * 128],
                                          tp[:])
                d_tiles[st] = (ot, oTst)
            ot, oTst = d_tiles[st]
            pd = pps.tile([128, 512], F32, tag="pp", name="pd")
            for vc in range(NJ):
                nc.tensor.matmul(
                    pd[:],
                    oTst[:, vc * 128:(vc + 1) * 128],
                    wo_t[:, vc * D + od * 512: vc * D + (od + 1) * 512],
                    start=(vc == 0), stop=(vc == NJ - 1))
            nc.vector.tensor_copy(ot[:, od * 512:(od + 1) * 512], pd[:])
            if od == 1:
                nc.sync.dma_start(out[st * 128:(st + 1) * 128, :], ot[:])
                del d_tiles[st]

        # ---- main loop: C(., sc) with B(sc+1) + D pieces as fillers ----
        # D s-tiles and part of B(3) are back-loaded into sc=3 (which has
        # no B(4) to fill its ACT-bound stretches with).
        d_sched = {0: [], 1: [0, 1], 2: [2, 3], 3: [4, 5, 6, 7, 8, 9, 10, 11]}
        # B groups deferred from sc=2 emission into sc=3 fillers: V st2/st3
        # (k-tiles 14/15, first needed by C(0,7)) and jt2/jt3 projections
        # (first needed by C(2,.)).
        b3_defer = [10, 11, 2, 6, 3, 7]
        for gi in range(12):
            b_group(0, gi)
        # filler sites per sc: (k-groups - 1) + 3 per (hp, qc)
        n_sites = {0: 24, 1: 40, 2: 48, 3: 56}
        for sc in range(NSC):
            if sc < NSC - 1:
                dma_xc(sc + 1)
                gis = range(12) if sc < NSC - 2 else \
                    [g for g in range(12) if g not in b3_defer]
                for gi in gis:
                    fillers.append(lambda sc=sc, gi=gi: b_group(sc + 1, gi))
            else:
                for gi in b3_defer:
                    fillers.append(lambda gi=gi: b_group(3, gi))
            for st in d_sched[sc]:
                for od in range(2):
                    fillers.append(lambda st=st, od=od: phase_d(st, od))
            fill_state["site"] = 0
            fill_state["stride"] = max(1, n_sites[sc] // max(1, len(fillers)))
            for hp in range(NJ):
                phase_c(hp, 2 * sc)
                pop_filler()
                phase_c(hp, 2 * sc + 1)
                pop_filler()
            while fillers:
                pop_filler(force=True)
        for st4 in range(4):
            phase_d(12 + st4)

    nc.compile()
    return nc


# ----------------------------------------------------------------- host side

_NC_CACHE = None


def _get_nc():
    global _NC_CACHE
    if _NC_CACHE is None:
        _NC_CACHE = build_mha()
    return _NC_CACHE


def make_masks():
    """[128, 512]: two diagonal masks for [128k x 256q] tiles.
    mask0: k-tile aligned with q-chunk start (keep k<=q: i<=j).
    mask128: k-tile offset +128 (keep i+128<=j)."""
    i = np.arange(128)[:, None]
    j = np.arange(QW)[None, :]
    m0 = (i <= j).astype(np.float32)
    m128 = (i + 128 <= j).astype(np.float32)
    return np.concatenate([m0, m128], axis=1)


def shard_inputs(x, Wq, bq, Wk, bk, Wv, bv, Wo, bo):
    import ml_dtypes
    masks = make_masks().astype(ml_dtypes.bfloat16)
    ident = np.eye(128, dtype=np.float32).astype(ml_dtypes.bfloat16)
    x = np.asarray(x, dtype=np.float32)
    Wq, Wk, Wv, Wo = (np.asarray(a, dtype=np.float32)
                      for a in (Wq, Wk, Wv, Wo))
    bq = np.asarray(bq, dtype=np.float32)
    maps = []
    for c in range(8):
        b, g = c // 2, c % 2
        sl = slice(g * JC, (g + 1) * JC)
        # bq per-core slice laid out [128 part, jt]: j = jt*128 + p
        bqt = np.ascontiguousarray(
            bq[sl].reshape(NJ, 128).T).astype(np.float32)
        maps.append({
            "xT": np.ascontiguousarray(x[b].T).astype(ml_dtypes.bfloat16),
            "wq": np.ascontiguousarray(Wq[:, sl]).astype(ml_dtypes.bfloat16),
            "wk": np.ascontiguousarray(Wk[:, sl]).astype(ml_dtypes.bfloat16),
            "wv": np.ascontiguousarray(Wv[:, sl]).astype(ml_dtypes.bfloat16),
            "wo": np.ascontiguousarray(Wo[sl, :]).astype(ml_dtypes.bfloat16),
            "bqt": bqt,
            "masks": masks,
            "ident": ident,
        })
    return maps


def kernel(x, Wq, bq, Wk, bk, Wv, bv, Wo, bo):
    """Full-input entry point: shard across 8 NeuronCores, run, gather."""
    from concourse.bass_utils import run_bass_kernel_spmd

    nc = _get_nc()
    in_maps = shard_inputs(x, Wq, bq, Wk, bk, Wv, bv, Wo, bo)
    res = run_bass_kernel_spmd(nc, in_maps, list(range(8)))
    # bv contributes bv @ Wo to every output row (softmax weights sum to 1)
    bo_eff = (np.asarray(bo, dtype=np.float32)
              + np.asarray(bv, dtype=np.float32)
              @ np.asarray(Wo, dtype=np.float32))
    out = np.empty((4, S, D), dtype=np.float32)
    for b in range(4):
        out[b] = res.results[2 * b]["out"] + res.results[2 * b + 1]["out"] \
            + bo_eff
    return out


# revision 81
# speedup vs baseline: 1.4855x; 1.1596x over previous
"""Bass/Tile MHA kernel for TRN2 — per-core program (v3).

Sharding (8 cores): core c handles batch b=c//2, head-group g=c%2
(8 of 16 heads).  Host pre-transposes x per batch to xT [1024, 2048]
and provides it (and the x32-scaled W_q/k/v head-group slices) as
fp8e4m3 hi/lo pairs for DoubleRow matmuls; folds bv into
bo' = bo + bv @ Wo (softmax weights sum to 1); drops bk entirely
(a key-side bias adds a per-query constant to every logit, which
softmax cancels).  The x32 weight scale keeps the fp8 lo-residual out
of the e4m3 subnormal floor; 1/1024 folds into the exp scale and 1/32
into wo.

Per core inputs:
  xh/xl [1024,2048] fp8, w{q,k,v}{h,l} [1024,512] fp8 (x32, col
  slice), wo [512,1024] bf16 (row slice, /32), bqt [128,4] f32 (x32),
  masks [128,512] bf16 (diag dd=0 | dd=128 for 256-wide q chunks),
  ident [128,128] bf16.  out [2048,1024] f32 partial (host sums TP
  pairs + bo').

Per-core schedule:
  B(sc): DMA x s-chunk -> qT[j,s] (+bq), kT[j,s], v pair blocks via
    fp8 DoubleRow (3 hi/lo products per dc-pair, 0.75 cyc/row/dc).
  C(hp,qc): per head pair / 256-q chunk / head: scores K^T.Q per
    128-k causal tile (bf16, masked diagonal tiles first, top tile's
    dead 128-q half skipped), exp (ACT -> bf16), attnV FLIPPED:
    po[q, V|1] with q on all 128 partitions so the softmax denominator
    lands on the numerators' partition -> per-partition recip+scale
    (no PE broadcast).
  D(st): PE-transpose o[q,dv] -> oT (bf16 identity), out = oT.wo.
  The loop interleaves B/D pieces into C's ACT-bound stretches via a
  credit-paced filler queue with data-dependency require() guards.
"""

from contextlib import ExitStack

import numpy as np

import concourse.bass as bass
import concourse.bacc as bacc
import concourse.mybir as mybir
import concourse.tile as tile

F32 = mybir.dt.float32
BF16 = mybir.dt.bfloat16
FP8 = mybir.dt.float8e4
DR = mybir.MatmulPerfMode.DoubleRow
ADD = mybir.AluOpType.add
MULT = mybir.AluOpType.mult
EXP = mybir.ActivationFunctionType.Exp

S = 2048          # sequence length
D = 1024          # model dim
JC = 512          # per-core projection width (8 heads * 64)
DK = 64           # head dim
NSC = 4           # s-chunks of 512
NST = 16          # s-tiles of 128
ND = 8            # d-chunks of 128
NJ = 4            # j-chunks of 128 (head pairs)
NQC = 8           # q-chunks of 256
QW = 256          # q-chunk width in phase C
VW = DK + 1       # 65: even-head attnV window (V + ones col)
PW = 160          # v pair block: [V_even(64)|ones(1)|gap(31)|V_odd(64)]


def build_mha():
    nc = bacc.Bacc("TRN2", target_bir_lowering=False, debug=False)

    xh = nc.dram_tensor("xh", [D, S], FP8, kind="ExternalInput").ap()
    xl = nc.dram_tensor("xl", [D, S], FP8, kind="ExternalInput").ap()
    wqh = nc.dram_tensor("wqh", [D, JC], FP8, kind="ExternalInput").ap()
    wql = nc.dram_tensor("wql", [D, JC], FP8, kind="ExternalInput").ap()
    wkh = nc.dram_tensor("wkh", [D, JC], FP8, kind="ExternalInput").ap()
    wkl = nc.dram_tensor("wkl", [D, JC], FP8, kind="ExternalInput").ap()
    wvh = nc.dram_tensor("wvh", [D, JC], FP8, kind="ExternalInput").ap()
    wvl = nc.dram_tensor("wvl", [D, JC], FP8, kind="ExternalInput").ap()
    wo = nc.dram_tensor("wo", [JC, D], BF16, kind="ExternalInput").ap()
    bqt = nc.dram_tensor("bqt", [128, NJ], F32, kind="ExternalInput").ap()
    masks = nc.dram_tensor("masks", [128, 2 * QW], BF16,
                           kind="ExternalInput").ap()
    ident = nc.dram_tensor("ident", [128, 128], BF16,
                           kind="ExternalInput").ap()
    out = nc.dram_tensor("out", [S, D], BF16,
                         kind="ExternalOutput").ap()

    xhd3 = xh.rearrange("(c p) s -> p c s", p=128)
    xld3 = xl.rearrange("(c p) s -> p c s", p=128)

    with tile.TileContext(nc) as tc, ExitStack() as ctx:
        const = ctx.enter_context(tc.tile_pool(name="const", bufs=1))
        bq_t = const.tile([128, NJ], F32)
        ident_t = const.tile([128, 128], BF16)
        mask_t = const.tile([128, 2 * QW], BF16)

        # persistent activations
        big = ctx.enter_context(tc.tile_pool(name="big", bufs=1))
        qT_t = big.tile([128, NJ * S], BF16, tag="qT")    # [j, s] 16KB/part
        kT_t = big.tile([128, NJ * S], BF16, tag="kT")    # [j, s] 16KB/part
        v_t = big.tile([128, NST * 4 * PW], BF16, tag="v")  # [s, pair*160]
        o_sb = big.tile([128, NST * NJ * 128], BF16, tag="o")  # [q,(qt hp d)]

        # psum pools (8 banks: pps 2*1 + psc 2*2 + pat 2*1)
        pps = ctx.enter_context(tc.tile_pool(name="pps", bufs=2, space="PSUM"))
        psc = ctx.enter_context(tc.tile_pool(name="psc", bufs=2, space="PSUM"))
        pat = ctx.enter_context(tc.tile_pool(name="pat", bufs=2, space="PSUM"))

        ep = ctx.enter_context(tc.tile_pool(name="exp", bufs=6))
        rp = ctx.enter_context(tc.tile_pool(name="recip", bufs=4))
        otp = ctx.enter_context(tc.tile_pool(name="otp", bufs=2))

        wp = ctx.enter_context(tc.tile_pool(name="wts", bufs=1))
        xin = ctx.enter_context(tc.tile_pool(name="xin", bufs=2))
        w_ts = {}
        for wname in ("wqh", "wql", "wkh", "wkl", "wvh", "wvl"):
            w_ts[wname] = wp.tile([128, ND * JC], FP8, tag=wname,
                                  name=wname)
        wo_t = wp.tile([128, NJ * D], BF16, tag="wo")

        ostp = ctx.enter_context(tc.tile_pool(name="ost", bufs=4))

        xc_tiles = {}

        def dma_xc(sc, half=None):
            if half is None or half == 0:
                xch = xin.tile([128, ND * 512], FP8, tag="xch", name="xch")
                xcl = xin.tile([128, ND * 512], FP8, tag="xcl", name="xcl")
                xc_tiles[sc] = (
                    xch[:].rearrange("p (c s) -> p c s", c=ND),
                    xcl[:].rearrange("p (c s) -> p c s", c=ND))
            hd = ND // 2
            halves = range(2) if half is None else (half,)
            for h in halves:
                for xcr, xd3 in zip(xc_tiles[sc], (xhd3, xld3)):
                    nc.sync.dma_start(
                        xcr[:, h * hd:(h + 1) * hd, :],
                        xd3[:, h * hd:(h + 1) * hd,
                            sc * 512:(sc + 1) * 512])

        def dma_w(wname, w_dram, half):
            hd = ND // 2
            nc.sync.dma_start(
                w_ts[wname][:].rearrange("p (c j) -> p c j", c=ND)[
                    :, half * hd:(half + 1) * hd, :],
                w_dram.rearrange("(c p) j -> p c j", p=128)[
                    :, half * hd:(half + 1) * hd, :])

        # ---- startup DMAs (emission order = fetch priority) ----
        dma_xc(0, half=0)
        dma_w("wqh", wqh, 0)
        dma_w("wql", wql, 0)
        dma_xc(0, half=1)
        dma_w("wqh", wqh, 1)
        dma_w("wql", wql, 1)
        for h in range(2):
            dma_w("wkh", wkh, h)
            dma_w("wkl", wkl, h)
        for h in range(2):
            dma_w("wvh", wvh, h)
            dma_w("wvl", wvl, h)
        nc.gpsimd.dma_start(bq_t[:], bqt[:, :])
        nc.gpsimd.dma_start(mask_t[:], masks[:, :])
        nc.gpsimd.dma_start(ident_t[:], ident[:, :])
        nc.sync.dma_start(wo_t[:].rearrange("p (c j) -> p c j", c=NJ),
                          wo.rearrange("(c p) j -> p c j", p=128))

        # v pair block: [V_even(64) | ones(1) | V_odd(64) | pad(31)]
        vr = v_t[:].rearrange("p (s q w) -> p s q w", s=NST, q=4)
        nc.vector.memset(vr[:, :, :, DK:DK + 1], 1.0)

        # ---- phase B pieces: 12 matmul groups x 2 halves per s-chunk ----
        # fp8 DoubleRow with hi/lo error compensation: each dc-PAIR takes 3
        # DR matmuls — (w_hi,x_hi)+(w_hi,x_hi) slots over both dcs, then
        # (w_lo,x_hi) and (w_hi,x_lo); the dropped lo*lo term is ~1e-3.
        b_open = {}

        def b_dr(pb_slice, prods, jlo, jw, nlo, nw):
            """Accumulate a [jw(<=128) x nw(<=256)] psum slice over 4
            dc-pairs x 3 hi/lo products with DoubleRow matmuls."""
            n = 0
            for dcp in range(4):
                dc0 = 2 * dcp
                for lA, rA in prods:
                    n += 1
                    nc.tensor.matmul(
                        pb_slice,
                        lA[:, dc0:dc0 + 2, jlo:jlo + jw],
                        rA[:, dc0:dc0 + 2, nlo:nlo + nw],
                        start=(n == 1), stop=(n == 12),
                        perf_mode=DR)

        def b_group(sc, gi, half=None):
            """Group gi in 0..11: 0-3 qT j-tiles, 4-7 kT j-tiles, 8-11 V.
            Halves 0/1 each cover a 256-wide output strip (12 DR matmuls,
            ~640ns PE), sized to the per-exp-group ACT deficit."""
            if half is None:
                b_group(sc, gi, 0)
                b_group(sc, gi, 1)
                return
            xch, xcl = xc_tiles[sc]
            if half == 0:
                b_open[(sc, gi)] = pps.tile([128, 512], F32, tag="pp",
                                            name="pb")
            pb = b_open[(sc, gi)]
            if gi < 8:
                wn = "wq" if gi < 4 else "wk"
                wh = w_ts[wn + "h"][:].rearrange("p (c j) -> p c j", c=ND)
                wl = w_ts[wn + "l"][:].rearrange("p (c j) -> p c j", c=ND)
                dstT = qT_t if gi < 4 else kT_t
                jt = gi % 4
                prods = [(wh, xch), (wl, xch), (wh, xcl)]
                b_dr(pb[:, half * 256:(half + 1) * 256], prods,
                     jt * 128, 128, half * 256, 256)
                if half == 1:
                    dst = dstT[:, jt * S + sc * 512: jt * S + (sc + 1) * 512]
                    if gi < 4:
                        nc.vector.tensor_scalar(
                            dst, pb[:], bq_t[:, jt:jt + 1], None, op0=ADD)
                    else:
                        nc.vector.tensor_copy(dst, pb[:])
            else:
                st4 = gi - 8
                st = sc * 4 + st4
                wh = w_ts["wvh"][:].rearrange("p (c j) -> p c j", c=ND)
                wl = w_ts["wvl"][:].rearrange("p (c j) -> p c j", c=ND)
                prods = [(xch, wh), (xcl, wh), (xch, wl)]
                b_dr(pb[:, half * 256:(half + 1) * 256], prods,
                     st4 * 128, 128, half * 256, 256)
                if half == 1:
                    pv4 = pb[:].rearrange("p (q two w) -> p q two w",
                                          q=4, two=2)
                    nc.vector.tensor_copy(vr[:, st, :, 0:DK],
                                          pv4[:, :, 0, :])
                    nc.vector.tensor_copy(vr[:, st, :, VW:VW + DK],
                                          pv4[:, :, 1, :])
            if half == 1:
                del b_open[(sc, gi)]

        # Filler scheduling uses a coarse build-time clock model: est["pe"]
        # is cumulative emitted PE-busy ns, est["act"] the projected ACT
        # completion time.  Fillers are popped exactly when ACT runs ahead,
        # so foreign PE work lands in the exp-bound stretches.
        fillers = []
        est = {"credit": 0.0, "rate": 0.0}
        CYC = 0.4167

        done_keys = set()

        def pop_filler(force=False):
            if force:
                if fillers:
                    key, fn, pe_ns = fillers.pop(0)
                    fn()
                    done_keys.add(key)
                return
            est["credit"] += est["rate"]
            while fillers and est["credit"] >= 1.0:
                est["credit"] -= 1.0
                key, fn, pe_ns = fillers.pop(0)
                fn()
                done_keys.add(key)

        def require(*keys):
            """Force-emit queued fillers until all `keys` have run (data
            dependencies of the upcoming phase_c block)."""
            while fillers and not all(k in done_keys for k in keys):
                pop_filler(force=True)

        def phase_c(hp, qc):
            """Attention for head pair hp, q-chunk qc (256 wide).

            Scores land as e[k, q]; attnV is flipped: out po[q, V|1] with q
            on the full 128 partitions (lhsT = e 128-q slice, rhs =
            [V|ones] 65-wide moving).  The softmax denominator then lands
            on the same partition as its numerators, so normalization is a
            per-partition reciprocal+scale — no PE broadcast needed.

            The top diagonal k-tile (kt = n_k-1) only covers the second
            128-q half (its first half is fully causal-masked), so it gets
            a 128-wide slot; slots are packed tightly into 1024-wide psum
            groups with one exp per group.
            """
            n_k = 2 * qc + 2
            # Masked diagonal tiles FIRST so their DVE mask-multiply is off
            # the block's critical tail; psum accumulation is order-free.
            order = [n_k - 2, n_k - 1] + list(range(n_k - 2))
            # (kt, offset, width) slots packed greedily into 1024-wide
            # groups; within a group wide slots go first so no matmul
            # crosses a 512-f32 psum bank boundary.
            groups, cur, w_acc = [], [], 0
            for kt in order:
                w = 128 if kt == n_k - 1 else QW
                if w_acc + w > 1024:
                    groups.append(cur)
                    cur, w_acc = [], 0
                cur.append((kt, w))
                w_acc += w
            groups.append(cur)
            g2 = []
            for g in groups:
                g = sorted(g, key=lambda s: -s[1])
                off, withoff = 0, []
                for kt, w in g:
                    withoff.append((kt, off, w))
                    off += w
                g2.append(withoff)
            groups = g2
            # po accumulation flags follow EMISSION order, not kt order
            eseq = [kt for g in groups for (kt, _, _) in g]
            emit = {0: [kt for kt in eseq if kt != n_k - 1], 1: eseq}
            emit_first = {qs: emit[qs][0] for qs in range(2)}
            emit_last = {qs: emit[qs][-1] for qs in range(2)}
            for h2 in range(2):
                lo = h2 * 64
                po = [pat.tile([128, VW], F32, tag="po", name=f"po{qs}")
                      for qs in range(2)]

                def attn_v(g, e):
                    """Masks + attnV matmuls for a score group."""
                    for kt, off, w in g:
                        if kt == n_k - 2:  # diagonal tile, mask0
                            nc.vector.tensor_tensor(
                                e[:, off:off + QW], e[:, off:off + QW],
                                mask_t[:, 0:QW], op=MULT)
                        elif kt == n_k - 1:  # top tile: right half, mask128
                            nc.vector.tensor_tensor(
                                e[:, off:off + 128], e[:, off:off + 128],
                                mask_t[:, QW + 128:QW + 256], op=MULT)
                        base = kt * 4 * PW + hp * PW + h2 * DK
                        for qs in ((1,) if w == 128 else (0, 1)):
                            nc.tensor.matmul(
                                po[qs][:],
                                e[:, off + qs * 128 - (QW - w):
                                  off + qs * 128 - (QW - w) + 128],
                                v_t[:, base: base + VW],
                                start=(kt == emit_first[qs]),
                                stop=(kt == emit_last[qs]))

                for g in groups:
                    gw = g[-1][1] + g[-1][2]
                    ps = psc.tile([128, 1024], F32, tag="sc", name="ps")
                    for kt, off, w in g:
                        qoff = hp * S + qc * QW + (QW - w)
                        nc.tensor.matmul(
                            ps[:, off:off + w],
                            kT_t[lo:lo + 64,
                                 hp * S + kt * 128: hp * S + (kt + 1) * 128],
                            qT_t[lo:lo + 64, qoff: qoff + w],
                            start=True, stop=True)
                    e = ep.tile([128, 1024], BF16, tag="e", name="e")
                    nc.scalar.activation(e[:, :gw], ps[:, :gw],
                                         EXP, scale=0.125 / 1024)
                    pop_filler()  # PE fills while ACT runs the exp
                    attn_v(g, e)
                # normalize: denominator is col DK (h0) / col 0 (h1)
                dcol, ncol = (DK, 0) if h2 == 0 else (0, 1)
                for qs in range(2):
                    qt = qc * 2 + qs
                    rc = rp.tile([128, 1], F32, tag="rec", name="rc")
                    nc.vector.reciprocal(rc[:], po[qs][:, dcol:dcol + 1])
                    nc.vector.tensor_scalar(
                        o_sb[:, (qt * NJ + hp) * 128 + lo:
                             (qt * NJ + hp) * 128 + lo + DK],
                        po[qs][:, ncol:ncol + DK], rc[:], None, op0=MULT)
                pop_filler()

        d_tiles = {}

        def phase_d(st, piece=None):
            """Pieces 0-3: (od, half) quarters.  Piece 0 also transposes
            o[q, dv2] -> oT[dv2, q] per head pair (PE, bf16 identity)."""
            if piece is None:
                for p in range(4):
                    phase_d(st, p)
                return
            od, half = piece // 2, piece % 2
            if piece == 0:
                ot = ostp.tile([128, D], BF16, tag="ost", name="ot")
                oTst = otp.tile([128, NJ * 128], BF16, tag="oTst",
                                name="oTst")
                for hp in range(NJ):
                    tp = psc.tile([128, 128], BF16, tag="sc", name="tp")
                    nc.tensor.transpose(
                        tp[:],
                        o_sb[:, (st * NJ + hp) * 128:
                             (st * NJ + hp + 1) * 128],
                        ident_t[:])
                    nc.vector.tensor_copy(oTst[:, hp * 128:(hp + 1) * 128],
                                          tp[:])
                d_tiles[st] = (ot, oTst, {})
            ot, oTst, pds = d_tiles[st]
            if half == 0:
                pds[od] = pps.tile([128, 512], F32, tag="pp", name="pd")
            pd = pds[od]
            for vc in (range(2) if half == 0 else range(2, NJ)):
                nc.tensor.matmul(
                    pd[:],
                    oTst[:, vc * 128:(vc + 1) * 128],
                    wo_t[:, vc * D + od * 512: vc * D + (od + 1) * 512],
                    start=(vc == 0), stop=(vc == NJ - 1))
            if half == 1:
                nc.vector.tensor_copy(ot[:, od * 512:(od + 1) * 512], pd[:])
                del pds[od]
            if piece == 3:
                nc.sync.dma_start(out[st * 128:(st + 1) * 128, :], ot[:])
                del d_tiles[st]

        # ---- main loop: C(., sc) with B + D pieces as fillers ----
        # Each B(s) chunk (s>=1) splits: the groups C(., 2s) touches first
        # (qT0/kT0/V st0/st1) emit during sc=s-1; the rest defer into sc=s
        # behind require() guards, spreading PE work into the late,
        # exp-bound stretches.  D s-tiles are back-loaded similarly.
        d_sched = {0: [], 1: [], 2: [0, 1, 2, 3, 4, 5],
                   3: [6, 7, 8, 9, 10, 11]}
        b_defer = [1, 5, 10, 11, 2, 6, 3, 7]
        for gi in range(12):
            b_group(0, gi)
        # filler sites per sc: one per exp group + one per (h2, hp, qc)
        n_sites = {0: 32, 1: 48, 2: 64, 3: 80}
        for sc in range(NSC):
            if sc >= 1:
                for gi in b_defer:
                    for half in range(2):
                        fillers.append((
                            ("b", sc, gi, half),
                            lambda sc=sc, gi=gi, half=half:
                            b_group(sc, gi, half), 0))
            if sc < NSC - 1:
                dma_xc(sc + 1)
                for gi in [g for g in range(12) if g not in b_defer]:
                    for half in range(2):
                        fillers.append((
                            ("b", sc + 1, gi, half),
                            lambda sc=sc, gi=gi, half=half:
                            b_group(sc + 1, gi, half), 0))
            for st in d_sched[sc]:
                for piece in range(4):
                    fillers.append((
                        ("d", st, piece),
                        lambda st=st, piece=piece: phase_d(st, piece), 0))
            est["credit"] = 0.0
            est["rate"] = len(fillers) / n_sites[sc]
            for qci, qc in enumerate((2 * sc, 2 * sc + 1)):
                for hp in range(NJ):
                    if sc >= 1:
                        req = [("b", sc, g, q) for g in (hp, 4 + hp)
                               for q in range(2) if g in b_defer]
                        if qci == 1:
                            req += [("b", sc, g, q) for g in (10, 11)
                                    for q in range(2)]
                        require(*req)
                    phase_c(hp, qc)
                if sc == NSC - 1 and qci == 0:
                    # q-tiles 12/13 are final after the qc=6 pass: their
                    # phase D becomes filler for the last C stretch
                    for st in (12, 13):
                        for piece in range(4):
                            fillers.append((
                                ("d", st, piece),
                                lambda st=st, piece=piece:
                                phase_d(st, piece), 0))
                    est["rate"] = len(fillers) / 40
            while fillers:
                pop_filler(force=True)
        for st4 in range(2):
            phase_d(14 + st4)

    nc.compile()
    return nc


# ----------------------------------------------------------------- host side

_NC_CACHE = None


def _get_nc():
    global _NC_CACHE
    if _NC_CACHE is None:
        _NC_CACHE = build_mha()
    return _NC_CACHE


def make_masks():
    """[128, 512]: two diagonal masks for [128k x 256q] tiles.
    mask0: k-tile aligned with q-chunk start (keep k<=q: i<=j).
    mask128: k-tile offset +128 (keep i+128<=j)."""
    i = np.arange(128)[:, None]
    j = np.arange(QW)[None, :]
    m0 = (i <= j).astype(np.float32)
    m128 = (i + 128 <= j).astype(np.float32)
    return np.concatenate([m0, m128], axis=1)


def split_fp8(a):
    """hi/lo decomposition: a ~= hi + lo with both in fp8e4m3."""
    import ml_dtypes
    f8 = ml_dtypes.float8_e4m3fn
    hi = a.astype(f8)
    lo = (a - hi.astype(np.float32)).astype(f8)
    return np.ascontiguousarray(hi), np.ascontiguousarray(lo)


def shard_inputs(x, Wq, bq, Wk, bk, Wv, bv, Wo, bo):
    import ml_dtypes
    masks = make_masks().astype(ml_dtypes.bfloat16)
    ident = np.eye(128, dtype=np.float32).astype(ml_dtypes.bfloat16)
    x = np.asarray(x, dtype=np.float32)
    Wq, Wk, Wv, Wo = (np.asarray(a, dtype=np.float32)
                      for a in (Wq, Wk, Wv, Wo))
    bq = np.asarray(bq, dtype=np.float32)
    maps = []
    for c in range(8):
        b, g = c // 2, c % 2
        sl = slice(g * JC, (g + 1) * JC)
        # bq per-core slice laid out [128 part, jt]: j = jt*128 + p
        # weights are pre-scaled x32 so their fp8 hi/lo split avoids the
        # e4m3 subnormal floor; 1/1024 folds into the exp scale and 1/32
        # into wo (the ones-column denominator is unscaled, so o_sb is x32).
        bqt = np.ascontiguousarray(
            bq[sl].reshape(NJ, 128).T).astype(np.float32) * 32.0
        xh, xl = split_fp8(np.ascontiguousarray(x[b].T))
        wqh, wql = split_fp8(Wq[:, sl] * 32.0)
        wkh, wkl = split_fp8(Wk[:, sl] * 32.0)
        wvh, wvl = split_fp8(Wv[:, sl] * 32.0)
        maps.append({
            "xh": xh, "xl": xl,
            "wqh": wqh, "wql": wql,
            "wkh": wkh, "wkl": wkl,
            "wvh": wvh, "wvl": wvl,
            "wo": np.ascontiguousarray(Wo[sl, :] / 32.0).astype(
                ml_dtypes.bfloat16),
            "bqt": bqt,
            "masks": masks,
            "ident": ident,
        })
    return maps


def kernel(x, Wq, bq, Wk, bk, Wv, bv, Wo, bo):
    """Full-input entry point: shard across 8 NeuronCores, run, gather."""
    from concourse.bass_utils import run_bass_kernel_spmd

    nc = _get_nc()
    in_maps = shard_inputs(x, Wq, bq, Wk, bk, Wv, bv, Wo, bo)
    res = run_bass_kernel_spmd(nc, in_maps, list(range(8)))
    # bv contributes bv @ Wo to every output row (softmax weights sum to 1)
    bo_eff = (np.asarray(bo, dtype=np.float32)
              + np.asarray(bv, dtype=np.float32)
              @ np.asarray(Wo, dtype=np.float32))
    out = np.empty((4, S, D), dtype=np.float32)
    for b in range(4):
        out[b] = (res.results[2 * b]["out"].astype(np.float32)
                  + res.results[2 * b + 1]["out"].astype(np.float32)
                  + bo_eff)
    return out
